# revision 30
# baseline (speedup 1.0000x reference)
"""BrokenBiasAttention Trainium2 kernel (8-core SPMD), v5.

Sharding: core c -> batch b=c//2, query-row-half r=c%2 (1024 of 2048 rows).

Structure (engine-balanced, merged-pair rounds):
  - Host precomputes BOTH bias tables in the final SBUF gather layout:
      expF = exp(bias)                       bf16  (ACT-path head pairs)
      schT = round(A16*(bias-20) + B16)     int16  (Schraudolph head pairs)
  - Schraudolph softmax for SCH_PAIRS: scores arrive pre-scaled by
    A16=128/ln2 (folded into host Wq columns); ONE DVE tensor_add
    (psum f32 + int16 table -> int16) yields bf16 bits of exp(s+b-20)
    directly (bitcast), replacing ACT exp + DVE multiply. Softmax
    normalization cancels the common-mode approx error (~6e-3 even with
    all heads approx).
  - Merged rounds: an ACT pair and a SCH pair are processed kt-by-kt
    together, scores 4-band row-packed across both pairs; the ACT
    engine (exp of pair a) and DVE (tensor_add of pair b) drain the two
    PSUM score tiles concurrently.
  - reciprocal_approx_fast epilogue, at partition base 0 only (base-64
    invocations of the custom DVE op corrupt results on HW).
  - PSUM-evacuation copies on the Scalar engine.
  - Table DMAs issued on the sync queue AFTER the x/w loads: the queue
    FIFO guarantees 8.6MB of table traffic cannot delay them.
"""

import math
import sys

import numpy as np

if "/opt/trn_rl_repo" not in sys.path:
    sys.path.insert(0, "/opt/trn_rl_repo")

N = 2048
C = 256
NH = 8
HD = 32
B = 4
QR = 1024  # q rows per core
S_SHIFT = 20.0
A16 = 128.0 / math.log(2.0)
B16 = 127.0 * 128.0

SCH_PAIRS = (3,)          # head pairs (g2) on the Schraudolph path
ACT_PAIRS = tuple(g for g in range(4) if g not in SCH_PAIRS)
# merged rounds: (pair_a ACT, pair_b) per qc; pairs must differ mod 2 so the
# four heads map to distinct 32-row PE bands
ROUNDS = [(0, 3), (1, 2)]

_NC = None


def _build_nc(dbg=False):
    import concourse.bass as bass
    import concourse.tile as tile
    from concourse import bacc, mybir
    from concourse.bass import ds, ts

    f32 = mybir.dt.float32
    bf16 = mybir.dt.bfloat16
    i16 = mybir.dt.int16
    EXP = mybir.ActivationFunctionType.Exp

    nA = len(ACT_PAIRS)
    nS = len(SCH_PAIRS)
    pair_slot = {}
    for j, g in enumerate(ACT_PAIRS):
        pair_slot[g] = j
    for j, g in enumerate(SCH_PAIRS):
        pair_slot[g] = j

    nc = bacc.Bacc(None, target_bir_lowering=False, debug=False)

    xT = nc.dram_tensor("xT", [C, N], bf16, kind="ExternalInput")
    xTq = nc.dram_tensor("xTq", [C, QR], bf16, kind="ExternalInput")
    Wq_d = nc.dram_tensor("Wq", [C, C], bf16, kind="ExternalInput")
    Wk_d = nc.dram_tensor("Wk", [C, C], bf16, kind="ExternalInput")
    Wv_d = nc.dram_tensor("Wv", [C, C], bf16, kind="ExternalInput")
    Wo_d = nc.dram_tensor("Wo", [C, C], bf16, kind="ExternalInput")
    expfT_d = None
    schT_d = None
    if nA:
        expfT_d = nc.dram_tensor("expfT", [128, nA * 8448], bf16, kind="ExternalInput")
    if nS:
        schT_d = nc.dram_tensor("schT", [128, nS * 8448], i16, kind="ExternalInput")
    out_d = nc.dram_tensor("out", [QR, C], f32, kind="ExternalOutput")
    dbg_t = {}
    if dbg:
        dbg_t["e_act"] = nc.dram_tensor("dbg_e_act", [128, 1024], bf16, kind="ExternalOutput")
        dbg_t["e_sch"] = nc.dram_tensor("dbg_e_sch", [128, 1024], i16, kind="ExternalOutput")

    with tile.TileContext(nc) as tc:
        with (
            tc.tile_pool(name="consts", bufs=1) as consts,
            tc.tile_pool(name="tbl", bufs=1) as tbl,
            tc.tile_pool(name="xp", bufs=3) as xp,
            tc.tile_pool(name="kqv", bufs=1) as kqv,
            tc.tile_pool(name="ep", bufs=14) as ep,
            tc.tile_pool(name="rp", bufs=4) as rp,
            tc.tile_pool(name="otp", bufs=2) as otp,
            tc.tile_pool(name="stp", bufs=2) as stp,
            tc.tile_pool(name="spsum", bufs=2, space="PSUM") as spsum,
            tc.tile_pool(name="apsum", bufs=4, space="PSUM") as apsum,
        ):
            expf_view = None
            sch_view = None
            if nA:
                expf_sb = tbl.tile([128, nA * 8448], bf16, tag="expf")
                expf_view = expf_sb.rearrange(
                    "p (h r f) -> p h r f", h=2 * nA, r=11, f=384
                )
            if nS:
                sch_sb = tbl.tile([128, nS * 8448], i16, tag="sch")
                sch_view = sch_sb.rearrange(
                    "p (h r f) -> p h r f", h=2 * nS, r=11, f=384
                )

            # ---- constants ----
            w_sb = {}
            for name, d in (("Wq", Wq_d), ("Wk", Wk_d), ("Wv", Wv_d), ("Wo", Wo_d)):
                t = consts.tile([128, 2, C], bf16, tag=f"w_{name}", name=f"w_{name}")
                nc.sync.dma_start(out=t, in_=d[:].rearrange("(ch p) n -> p ch n", p=128))
                w_sb[name] = t
            ones_sb = consts.tile([128, 32], bf16, tag="ones")
            nc.vector.memset(ones_sb, 1.0)
            ebias = consts.tile([128, 1], f32, tag="ebias")
            nc.vector.memset(ebias, -S_SHIFT)

            # ---- projections (all bf16; q scale folded into host Wq) ----
            kT_sb = [kqv.tile([128, N], bf16, tag=f"kT{m}", name=f"kT{m}")
                     for m in range(2)]
            qT_sb = [kqv.tile([128, QR], bf16, tag=f"qT{m}", name=f"qT{m}")
                     for m in range(2)]
            v_sb = kqv.tile([128, 16, C], bf16, tag="v")

            xTq_r = xTq[:].rearrange("(ch p) n -> p ch n", p=128)
            for j in range(QR // 512):
                xq = xp.tile([128, 2, 512], bf16, tag="x")
                nc.sync.dma_start(out=xq, in_=xTq_r[:, :, ds(512 * j, 512)])
                for m in range(2):
                    ps = spsum.tile([128, 1024], f32, tag="s")
                    for ch in range(2):
                        nc.tensor.matmul(
                            ps[:, :512],
                            lhsT=w_sb["Wq"][:, ch, ts(m, 128)],
                            rhs=xq[:, ch, :],
                            start=(ch == 0),
                            stop=(ch == 1),
                        )
                    nc.scalar.copy(qT_sb[m][:, ds(512 * j, 512)], ps[:, :512])

            xT_r = xT[:].rearrange("(ch p) n -> p ch n", p=128)
            for j in range(N // 512):
                xc = xp.tile([128, 2, 512], bf16, tag="x")
                nc.sync.dma_start(out=xc, in_=xT_r[:, :, ds(512 * j, 512)])
                for m in range(2):
                    ps = spsum.tile([128, 1024], f32, tag="s")
                    for ch in range(2):
                        nc.tensor.matmul(
                            ps[:, :512],
                            lhsT=w_sb["Wk"][:, ch, ts(m, 128)],
                            rhs=xc[:, ch, :],
                            start=(ch == 0),
                            stop=(ch == 1),
                        )
                    nc.scalar.copy(kT_sb[m][:, ds(512 * j, 512)], ps[:, :512])
                for t in range(4):
                    kt = 4 * j + t
                    ps = spsum.tile([128, 1024], f32, tag="s")
                    for ch in range(2):
                        nc.tensor.matmul(
                            ps[:, :C],
                            lhsT=xc[:, ch, ts(t, 128)],
                            rhs=w_sb["Wv"][:, ch, :],
                            start=(ch == 0),
                            stop=(ch == 1),
                        )
                    nc.scalar.copy(v_sb[:, kt, :], ps[:, :C])

            # ---- bias tables AFTER x/w on the SAME (sync) queue: the HW
            # queue FIFO guarantees they cannot delay the loads above
            for g2t in (0, 3, 1, 2):
                j = pair_slot[g2t]
                if g2t in SCH_PAIRS:
                    src, dst = schT_d, sch_sb
                else:
                    src, dst = expfT_d, expf_sb
                nc.sync.dma_start(
                    out=dst[:, ds(j * 8448, 8448)],
                    in_=src[:, ds(j * 8448, 8448)],
                )

            # ---- main attention: merged-pair rounds ----
            oT_tiles = []
            for qc in range(2):
                oT = otp.tile([128, 2, 512], bf16, tag="oT", name=f"oT{qc}")
                oT_tiles.append(oT)

            def emit_wo(qc):
                oTw = oT_tiles[qc]
                for s in range(4):
                    fps = spsum.tile([128, 1024], f32, tag="s")
                    for ch in range(2):
                        nc.tensor.matmul(
                            fps[:, :C],
                            lhsT=oTw[:, ch, ts(s, 128)],
                            rhs=w_sb["Wo"][:, ch, :],
                            start=(ch == 0),
                            stop=(ch == 1),
                        )
                    stage = stp.tile([128, C], f32, tag="stage")
                    nc.scalar.copy(stage, fps[:, :C])
                    nc.sync.dma_start(
                        out=out_d[ds(512 * qc + 128 * s, 128), :], in_=stage
                    )

            for qc in range(2):
                oT = oT_tiles[qc]
                for g2a, g2b in ROUNDS:
                    parts = []
                    for g2 in (g2a, g2b):
                        parts.append({
                            "g2": g2,
                            "sch": g2 in SCH_PAIRS,
                            "slot": pair_slot[g2],
                            "po_av": 0 if g2 % 2 == 0 else 64,
                            "po_rs": 64 if g2 % 2 == 0 else 0,
                            "half": g2 // 2,
                            "acc": apsum.tile([128, 512], f32, tag="acc",
                                              name=f"acc{g2}_{qc}"),
                            "e": {},
                        })

                    def emit_av(P, kt):
                        e_t, e_sch = P["e"].pop(kt)
                        for k in range(2):
                            h = 2 * P["g2"] + k
                            rhs = e_t[:, ts(k, 512)]
                            if e_sch:
                                rhs = rhs.bitcast(bf16)
                            nc.tensor.matmul(
                                P["acc"][ds(P["po_av"] + 32 * k, 32), :],
                                lhsT=v_sb[:, kt, ds(32 * h, 32)],
                                rhs=rhs,
                                start=(kt == 0),
                                stop=(kt == 15),
                                tile_position=(0, P["po_av"] + 32 * k),
                                skip_group_check=True,
                            )
                            nc.tensor.matmul(
                                P["acc"][ds(P["po_rs"] + 32 * k, 32), :],
                                lhsT=ones_sb,
                                rhs=rhs,
                                start=(kt == 0),
                                stop=(kt == 15),
                                tile_position=(0, P["po_rs"] + 32 * k),
                                skip_group_check=True,
                            )

                    for kt in range(16):
                        rdw0 = 2 * qc - (kt // 2) + 7
                        woff = 128 if kt % 2 == 0 else 0
                        sps = []
                        for P in parts:
                            s_ps = spsum.tile([128, 1024], f32, tag="s")
                            sps.append(s_ps)
                        # 4-band row-packed scores for both pairs
                        for P, s_ps in zip(parts, sps):
                            for k in range(2):
                                h = 2 * P["g2"] + k
                                i = h % 4
                                nc.tensor.matmul(
                                    s_ps[:, ts(k, 512)],
                                    lhsT=kT_sb[P["half"]][ds(32 * i, 32), ts(kt, 128)],
                                    rhs=qT_sb[P["half"]][ds(32 * i, 32), ts(qc, 512)],
                                    start=True,
                                    stop=True,
                                    tile_position=(32 * i, 0),
                                )
                        # consumers: SCH first (DVE), then ACT exp + DVE mul
                        for P, s_ps in zip(parts, sps):
                            if P["sch"]:
                                e_sb = ep.tile([128, 1024], i16, tag="e")
                                e4 = e_sb.rearrange("p (k jj f) -> p k jj f",
                                                    k=2, jj=2)
                                s4 = s_ps.rearrange("p (k jj f) -> p k jj f",
                                                    k=2, jj=2)
                                bt4 = sch_view[
                                    :, ds(2 * P["slot"], 2), ds(rdw0, 2),
                                    ds(woff, 256)
                                ]
                                nc.vector.tensor_add(e4, s4, bt4)
                                if dbg and qc == 0 and kt == 0 and P["g2"] == SCH_PAIRS[0]:
                                    nc.sync.dma_start(out=dbg_t["e_sch"][:], in_=e_sb)
                                P["e"][kt] = (e_sb, True)
                            else:
                                e_sb = ep.tile([128, 1024], bf16, tag="e")
                                nc.scalar.activation(e_sb, s_ps, EXP, bias=ebias[:, :])
                                ev = e_sb.rearrange("p (k jj f) -> p k jj f",
                                                    k=2, jj=2)
                                fv = expf_view[
                                    :, ds(2 * P["slot"], 2), ds(rdw0, 2),
                                    ds(woff, 256)
                                ]
                                nc.vector.tensor_mul(ev, ev, fv)
                                if dbg and qc == 0 and kt == 0 and P["g2"] == 0:
                                    nc.scalar.dma_start(out=dbg_t["e_act"][:], in_=e_sb)
                                P["e"][kt] = (e_sb, False)
                        if kt in (5, 8, 11, 14):
                            for P in parts:
                                for k2 in range(kt - 5, kt - 2):
                                    emit_av(P, k2)
                    for P in parts:
                        for k2 in (12, 13, 14, 15):
                            emit_av(P, k2)
                    # epilogues for both pairs
                    for P in parts:
                        po_av, po_rs = P["po_av"], P["po_rs"]
                        acc = P["acc"]
                        recip = rp.tile([128, 512], f32, tag="recip")
                        rep = rp.tile([128, 512], f32, tag="rep")
                        if po_rs == 0:
                            nc.vector.reciprocal_approx_fast(
                                recip[ds(0, 64), :], acc[ds(0, 64), :]
                            )
                        else:
                            nc.vector.tensor_copy(
                                rep[ds(64, 64), :], acc[ds(64, 64), :]
                            )
                            nc.sync.dma_start(
                                out=rep[ds(0, 64), :], in_=rep[ds(64, 64), :]
                            )
                            nc.vector.reciprocal_approx_fast(
                                recip[ds(0, 64), :], rep[ds(0, 64), :]
                            )
                        if po_av == 0:
                            nc.vector.tensor_mul(
                                oT[ds(0, 64), P["half"], :],
                                acc[ds(0, 64), :],
                                recip[ds(0, 64), :],
                            )
                        else:
                            nc.sync.dma_start(
                                out=rep[ds(64, 64), :], in_=recip[ds(0, 64), :]
                            )
                            nc.vector.tensor_mul(
                                oT[ds(64, 64), P["half"], :],
                                acc[ds(64, 64), :],
                                rep[ds(64, 64), :],
                            )
                if qc == 0:
                    emit_wo(0)
            emit_wo(1)

    nc.compile()
    return nc


def _host_tables(T):
    """Per-row-half bias tables in the final SBUF gather layout.

    Returns {r: (expf bf16 [128, nA*8448], sch int16 [128, nS*8448])}.
    Layout: partition p = 16*h2p + w2, free = (pair-slot-local head, rdw 11,
    f 384) where f = 16*drh + w1, gathered value
    G[p,h,rdw,f] = bias_table[h, 4r+rdw, (7-h2p)+drh, 15+w1-w2].
    """
    import ml_dtypes

    bf = ml_dtypes.bfloat16
    T = np.asarray(T, dtype=np.float32)
    p = np.arange(128)
    h2p, w2 = p // 16, p % 16
    f = np.arange(384)
    drh, w1 = f // 16, f % 16
    rh = (7 - h2p)[:, None] + drh[None, :]          # [128, 384]
    rw = 15 + w1[None, :] - w2[:, None]             # [128, 384]
    out = {}
    for r in (0, 1):
        Twin = T[:, 4 * r:4 * r + 11]               # [8, 11, 31, 31]
        G = Twin[:, :, rh, rw]                      # [8, 11, 128, 384]
        G = np.ascontiguousarray(G.transpose(2, 0, 1, 3))  # [128, 8, 11, 384]
        expf = None
        sch = None
        if ACT_PAIRS:
            heads = []
            for g in ACT_PAIRS:
                heads += [2 * g, 2 * g + 1]
            expf = np.ascontiguousarray(
                np.exp(G[:, heads]).reshape(128, -1).astype(bf)
            )
        if SCH_PAIRS:
            heads = []
            for g in SCH_PAIRS:
                heads += [2 * g, 2 * g + 1]
            sch = np.ascontiguousarray(
                np.round(A16 * (G[:, heads] - S_SHIFT) + B16)
                .reshape(128, -1).astype(np.int16)
            )
        out[r] = (expf, sch)
    return out


def _host_inputs(x, Wq, Wk, Wv, Wo, bias_table):
    """Build the 8 per-core input maps."""
    import ml_dtypes

    bf = ml_dtypes.bfloat16
    x = np.asarray(x, dtype=np.float32)
    xf = np.ascontiguousarray(x.reshape(B, N, C))
    qsc = 1.0 / math.sqrt(HD)
    scale = np.full(NH, qsc, np.float32)
    for g in SCH_PAIRS:
        scale[2 * g] = qsc * A16
        scale[2 * g + 1] = qsc * A16
    Wq_s = np.asarray(Wq, np.float32).reshape(C, NH, HD) * scale[None, :, None]
    Ws = {
        "Wq": np.ascontiguousarray(Wq_s.reshape(C, C).astype(bf)),
        "Wk": np.ascontiguousarray(np.asarray(Wk, np.float32).astype(bf)),
        "Wv": np.ascontiguousarray(np.asarray(Wv, np.float32).astype(bf)),
        "Wo": np.ascontiguousarray(np.asarray(Wo, np.float32).astype(bf)),
    }
    tables = _host_tables(bias_table)
    in_maps = []
    for c in range(8):
        b, r = c // 2, c % 2
        expf, sch = tables[r]
        m = {
            "xT": np.ascontiguousarray(xf[b].T.astype(bf)),
            "xTq": np.ascontiguousarray(xf[b, QR * r:QR * (r + 1)].T.astype(bf)),
            **Ws,
        }
        if expf is not None:
            m["expfT"] = expf
        if sch is not None:
            m["schT"] = sch
        in_maps.append(m)
    return in_maps


def kernel(x, Wq, Wk, Wv, Wo, bias_table, _results_hook=None):
    global _NC
    if _NC is None:
        _NC = _build_nc()
    from concourse.bass_utils import run_bass_kernel_spmd

    in_maps = _host_inputs(x, Wq, Wk, Wv, Wo, bias_table)
    res = run_bass_kernel_spmd(_NC, in_maps, core_ids=list(range(8)))
    if _results_hook is not None:
        _results_hook(res)
    out = np.zeros((B, N, C), dtype=np.float32)
    for c in range(8):
        b, r = c // 2, c % 2
        out[b, QR * r:QR * (r + 1)] = res.results[c]["out"]
    D, H, W = 8, 16, 16
    return out.reshape(B, D, H, W, C)
